# revision 45
# baseline (speedup 1.0000x reference)
"""Masked-softmax cross-entropy loss on 8 Trainium2 cores.

Math per target row t (16384 rows of length 4096):
  numer[t] = sum_j exp(x[t,j]/tau) over valid src cols j with color == tgt color t
  denom[t] = sum_j exp(x[t,j]/tau) over valid src cols j
  p_gt = numer/denom, nll = -log(p_gt + eps); rows with numer==0 masked out.

Device strategy: colors are mapped to small integer ids on host; the
per-color aggregation is a one-hot matmul on the otherwise-idle PE engine:
  bucket[k, t] = sum_j onehot[j, k] * exp(10*x^T[j, t])
x is pre-transposed and bf16-cast on host so j (the contraction dim) lands
on partitions and DMA bytes are halved.  The one-hot matrix is built on
device by the DVE (is_equal of per-chunk src ids against an iota row); pad
colors collect in row 127 and the host sums rows 0..126 for the
denominator.  ScalarE's exp is the critical engine (~59us busy, in-place
over each freshly-DMA'd tile); PE accumulates into 4 PSUM banks ([128
colors, 512 t] each); finished banks are cast to bf16 and DMA'd out while
later banks still accumulate.  Host gathers numer = bucket[tgt_id[t], t]
and finishes the tiny [B, 4096] reduction exactly as the reference does.

Every instruction carries at most ONE sync wait (this walrus rejects two):
nothing recycles (one SBUF slot per item), exp runs in place, and small
"touch" matmuls absorb DMA-lane/DVE ticks into PE's observed clock.

Sharding: core c takes batch c//2, row-half c%2 (2048 target rows).
"""

import os
import numpy as np

B = 4
S_TGT = 8
L_TGT = 512
C = 4
N = 4096          # src columns (= 8*512), also total tgt rows per batch
P = 128
ROWS = 2048       # tgt rows per core (half a batch)
NCHUNK = N // P   # 32 j-chunks of 128 src columns
# chunks per iteration: small first iterations cut the DMA ramp before the
# first exp (each 2MB transfer takes ~5.8us vs ~7us of exp it feeds); small
# last ones shrink the PE drain after the final exp.  The first and last
# entries are additionally split in half along t (banks 0-1 vs banks 2-3):
# the first exp starts off a quarter-size transfer, and banks 0-1 drain
# while banks 2-3 are still accumulating.
# items: (kbase, nchunks, t_lo, t_hi).  Chunk 0 in t-quarters and chunk 1
# in t-halves so the first exps ride tiny transfers; the last chunk in
# t-halves so banks 0-1 drain while banks 2-3 still accumulate.
ITEMS = (
    (0, 1, 0, 1024), (0, 1, 1024, 2048),
    (1, 1, 0, 2048),
    (2, 2, 0, 2048), (4, 2, 0, 2048),
    (6, 3, 0, 2048), (9, 3, 0, 2048),
    (12, 4, 0, 2048), (16, 4, 0, 2048), (20, 4, 0, 2048),
    (24, 4, 0, 2048),
    (28, 2, 0, 2048), (30, 1, 0, 2048),
    (31, 1, 0, 1024), (31, 1, 1024, 2048),
)
assert sum(s * (t1 - t0) for _, s, t0, t1 in ITEMS) == NCHUNK * ROWS
LDW_OPT = False   # walrus rejects explicit InstLdweights with ldw-opt
NBANK = ROWS // 512   # 4 psum banks: [128 colors, 512 t] each
KVALID = 127      # onehot column holding the valid-src indicator (denom)
NCORES = 8
PAD = -1.0
EPS = 1e-15

_NC_CACHE = {}


def _patch_split_drain():
    """Split the kernel-tail drain's sem waits across several drain
    instructions (walrus rejects >1 sync wait on one CTRL instruction)."""
    import concourse.tile as tile
    from concourse.vector_clock import ScopedClock, VectorClock

    if getattr(tile.TileContext, "_split_drain_patched", False):
        return

    def _drain_and_barrier(self, tick_clock, wait_clock):
        g = tick_clock.global_clock
        n = len(g)
        for base in range(n):
            vec = [g[i] if i == base else 0 for i in range(n)]
            if not any(vec):
                continue
            d = self.nc.sync.drain()
            wait_clock.add_sem_waits(d.ins, ScopedClock({None: VectorClock(vec)}))
        self.nc.all_engine_barrier()
        popped = self.nc._tile_sem_poison_stack.pop()
        assert popped is self._sem_poison
        self.nc.clear_and_free_semaphores(list(self.sems.allocated().values()))
        self.nc.all_engine_barrier()

    tile.TileContext._drain_and_barrier = _drain_and_barrier
    tile.TileContext._split_drain_patched = True


def _patch_ldw_dedup():
    """Drop InstLdweights that reload the PE array with the exact same
    stationary tile as the immediately preceding InstLdweights (we emit 4
    same-weight matmuls per contraction chunk; tile_legalize splits each
    into ldweights+matmult pairs).  The PE array persists across matmuls,
    so the repeats only cost ~110ns each plus a ~170ns pipeline restart on
    every following matmul.  Consumer deps are remapped to the kept load."""
    import concourse.tile as tile
    import concourse.mybir as mybir

    if getattr(tile, "_ldw_dedup_patched", False):
        return
    orig = tile.tile_legalize

    def _key(ins):
        try:
            return " ".join(str(a) for a in ins.ins)
        except Exception:
            return None

    def legalize_dedup(ordered, nc):
        out = orig(ordered, nc)
        res = {}
        for bb, insts in out.items():
            new = []
            remap = {}
            kept_name = None
            kept_key = None
            for ins in insts:
                if remap:
                    ins.remap_dependency_names(remap)
                if isinstance(ins, mybir.InstLdweights):
                    k = _key(ins)
                    if k is not None and k == kept_key and kept_name:
                        remap[ins.name] = kept_name
                        continue
                    kept_name, kept_key = ins.name, k
                elif not isinstance(ins, mybir.InstMatmult):
                    # any other instruction type: stop extending the run
                    # (engine-order reasoning only holds for matmul chains)
                    kept_name, kept_key = None, None
                new.append(ins)
            res[bb] = new
        return res

    tile.tile_legalize = legalize_dedup
    tile._ldw_dedup_patched = True


def _patch_ldw_opt():
    """Re-enable walrus's LDWEIGHTS dedup: adjacent matmuls sharing the same
    stationary tile then skip the redundant reload and its ~170ns pipeline
    restart (we emit 4 same-weight matmuls per contraction chunk)."""
    import concourse.bass_utils as bu

    if getattr(bu, "_ldw_opt_patched", False):
        return
    orig = bu.run_command

    def run_command_ldw(cmd, *args, **kwargs):
        if isinstance(cmd, list):
            cmd = [
                "--enable-ldw-opt=true" if c == "--enable-ldw-opt=false" else c
                for c in cmd
            ]
        return orig(cmd, *args, **kwargs)

    bu.run_command = run_command_ldw
    bu._ldw_opt_patched = True


def _build_nc():
    import concourse.bass as bass
    import concourse.mybir as mybir
    import concourse.tile as tile
    from contextlib import ExitStack

    _patch_split_drain()
    if LDW_OPT:
        _patch_ldw_opt()
    nc = bass.Bass()
    f32 = mybir.dt.float32
    bf16 = mybir.dt.bfloat16

    # host packs each item's chunks as a flat [P * (w + 1)] block
    # (last column junk, written only by the DMA -- see touch matmul)
    TOT = sum(P * (s * (t1 - t0) + 1) for _, s, t0, t1 in ITEMS)
    x = nc.declare_dram_parameter("x", [TOT], bf16, isOutput=False)
    # sid[:, 0:32] = per-chunk src color ids (pad = -1), sid[:, 32:160] =
    # iota row 0..126 then -1, identical on every partition
    sid = nc.declare_dram_parameter("sid", [P, 160], bf16, isOutput=False)
    bucket = nc.declare_dram_parameter("bucket", [P, ROWS], bf16, isOutput=True)

    with tile.TileContext(nc) as tc:
        with ExitStack() as ctx:
            from concourse.tile_rust import add_dep_helper

            const_pool = ctx.enter_context(tc.tile_pool(name="const", bufs=1))
            x_pool = ctx.enter_context(tc.tile_pool(name="x", bufs=1))
            psum_pool = ctx.enter_context(
                tc.tile_pool(name="psum", bufs=1, space="PSUM")
            )

            # two psum pair-tiles of two banks each: one cast + one store
            # DMA per pair instead of four of each
            pairs = [
                psum_pool.tile([P, 1024], f32, name=f"pair{i}", tag=f"pair{i}")
                for i in range(NBANK // 2)
            ]
            banks = [pairs[i // 2][:, (i % 2) * 512:(i % 2 + 1) * 512]
                     for i in range(NBANK)]
            junk = psum_pool.tile([1, 1], f32, name="junk", tag="junk")

            xts, offs = [], []
            off = 0
            for it, (kb, s, t0, t1) in enumerate(ITEMS):
                w = s * (t1 - t0)
                xts.append(x_pool.tile([P, w + 1], bf16, name=f"x{it}",
                                       tag=f"x{it}"))
                offs.append(off)
                off += P * (w + 1)

            # first x block, then the tiny sid DMA; the one-hot matrix is
            # built by the otherwise-idle DVE (bucket row k collects color
            # k, row 127 the pad mass; the host sums rows 0..126 for the
            # denominator).  All DMAs issue from the SP sequencer: issuing
            # some from the Activation sequencer delayed those transfers
            # by ~4us (measured), starving the exp stream.
            nc.sync.dma_start(
                xts[0][:],
                x[0:P * (ITEMS[0][1] * (ITEMS[0][3] - ITEMS[0][2]) + 1)]
                .rearrange("(p w) -> p w", p=P),
            )
            sidt = const_pool.tile([P, 160], bf16)
            nc.sync.dma_start(sidt[:], sid[:])
            mtt = const_pool.tile([P, N], bf16, name="mtt", tag="mtt")
            for kch in range(NCHUNK):
                nc.vector.tensor_tensor(
                    mtt[:, kch * P:(kch + 1) * P],
                    sidt[:, kch:kch + 1].to_broadcast((P, P)),
                    sidt[:, 32:160],
                    mybir.AluOpType.is_equal,
                )

            for it, (kb, s, t0, t1) in enumerate(ITEMS):
                tw = t1 - t0
                w = s * tw
                xt = xts[it]
                if it > 0:
                    nc.sync.dma_start(
                        xt[:],
                        x[offs[it]:offs[it] + P * (w + 1)].rearrange(
                            "(p w) -> p w", p=P
                        ),
                    )
                # exp in place over all but the junk column: fresh slot per
                # item, so this carries only the DMA wait
                nc.scalar.activation(
                    xt[:, 0:w], xt[:, 0:w],
                    mybir.ActivationFunctionType.Exp, scale=10.0,
                )
                # touch matmuls absorb one sync wait each into PE's observed
                # clock (walrus allows a single sync wait per instruction):
                # touch_x the x DMA lane via the junk column, touch_m the DVE
                # tick of this item's one-hot chunks; the real matmuls below
                # then need only the ACT wait
                touch_x = nc.tensor.matmul(
                    junk[:], xt[:, w:w + 1], xt[:, w:w + 1],
                    start=True, stop=True,
                )
                mcol = (kb + s) * P - 1
                touch_m = nc.tensor.matmul(
                    junk[:], mtt[:, mcol:mcol + 1], mtt[:, mcol:mcol + 1],
                    start=True, stop=True,
                )
                add_dep_helper(
                    touch_m.ins, touch_x.ins, sync=False,
                    reason="keep PE order",
                )
                prev = touch_m
                for cc in range(s):
                    kch = kb + cc
                    for nb in range(t0 // 512, t1 // 512):
                        mm = nc.tensor.matmul(
                            banks[nb][:],
                            mtt[:, kch * P:(kch + 1) * P],
                            xt[:, cc * tw + nb * 512 - t0:
                               cc * tw + (nb + 1) * 512 - t0],
                            start=(kch == 0),
                            stop=(kch == NCHUNK - 1 and cc == s - 1),
                        )
                        add_dep_helper(
                            mm.ins, prev.ins, sync=False,
                            reason="keep PE order: touch first",
                        )
                        prev = mm
                        if kch == NCHUNK - 1 and cc == s - 1 and nb % 2 == 1:
                            # pair complete: drain while the next pair's
                            # banks are still accumulating
                            pr = nb // 2
                            res = const_pool.tile(
                                [P, 1024], bf16, name=f"res{pr}", tag=f"res{pr}"
                            )
                            nc.vector.tensor_copy(res[:], pairs[pr][:])
                            nc.gpsimd.dma_start(
                                bucket[:, pr * 1024:(pr + 1) * 1024], res[:]
                            )
    return nc


def _get_nc():
    key = ITEMS
    if key not in _NC_CACHE:
        _NC_CACHE[key] = _build_nc()
    return _NC_CACHE[key]


def _color_ids(src, tgt):
    """Map each color row to a per-batch integer id via exact byte equality."""
    src_f = np.ascontiguousarray(src.reshape(B, -1, C))
    tgt_f = np.ascontiguousarray(tgt.reshape(B, -1, C))
    n_s = src_f.shape[1]
    src_ids = np.empty((B, n_s), np.int32)
    tgt_ids = np.empty((B, tgt_f.shape[1]), np.int32)
    for b in range(B):
        allc = np.ascontiguousarray(np.concatenate([src_f[b], tgt_f[b]], axis=0))
        view = allc.view([("", allc.dtype)] * C).reshape(-1)
        _, inv = np.unique(view, return_inverse=True)
        ids = inv.astype(np.int32)
        s_ids, t_ids = ids[:n_s].copy(), ids[n_s:].copy()
        s_ids[np.all(src_f[b] == PAD, axis=-1)] = -1
        t_ids[np.all(tgt_f[b] == PAD, axis=-1)] = -2
        src_ids[b], tgt_ids[b] = s_ids, t_ids
    return src_ids, tgt_ids


def kernel(seg_sim_map, seg_colors_src, seg_colors_tgt):
    import ml_dtypes
    from concourse.bass_utils import run_bass_kernel_spmd

    bf16 = ml_dtypes.bfloat16
    seg_sim_map = np.asarray(seg_sim_map, dtype=np.float32)
    src_ids, tgt_ids = _color_ids(
        np.asarray(seg_colors_src, np.float32), np.asarray(seg_colors_tgt, np.float32)
    )
    assert src_ids.max() <= KVALID - 1 and tgt_ids.max() <= KVALID - 1

    in_maps = []
    for c in range(NCORES):
        b, h = c // 2, c % 2
        # x^T for this core, packed per iteration as flat blocks of
        # [P, w+1] where [p, c*ROWS + t] = x[b, h*ROWS + t, (kbase+c)*P + p]
        xb = seg_sim_map[b].astype(bf16)
        xT = xb[h * ROWS:(h + 1) * ROWS, :].T          # [N j, ROWS t]
        xc = xT.reshape(NCHUNK, P, ROWS)               # [chunk, p, t]
        blocks = []
        for kb, s, t0, t1 in ITEMS:
            tw = t1 - t0
            w = s * tw
            blk = np.empty((P, w + 1), bf16)
            # chunks kb..kb+s side by side in the free dim, t-range [t0,t1)
            blk[:, :w] = (
                xc[kb:kb + s, :, t0:t1].transpose(1, 0, 2).reshape(P, w)
            )
            blk[:, w] = 0
            blocks.append(blk.reshape(-1))
        xt = np.concatenate(blocks)
        # per-chunk src ids + iota row for the on-device one-hot build
        sid = np.empty((P, 160), np.float32)
        sid[:, 0:NCHUNK] = src_ids[b].reshape(NCHUNK, P).T
        sid[:, NCHUNK:32] = -3.0          # unused slots, match nothing
        sid[:, 32:159] = np.arange(KVALID, dtype=np.float32)[None, :]
        sid[:, 159] = -1.0                # pad colors collect in row 127
        in_maps.append({"x": xt, "sid": sid.astype(bf16)})

    trace = os.environ.get("KERNEL_PROFILE", "") == "1"
    nc = _get_nc()
    out = run_bass_kernel_spmd(nc, in_maps, list(range(NCORES)), trace=trace)
    if trace and out.exec_time_ns is not None:
        print(f"HW exec time: {out.exec_time_ns} ns")
        print(f"HW exec mean: {out.mean_exec_time_ns} ns")

    numer = np.empty((B, N), np.float32)
    denom = np.empty((B, N), np.float32)
    for c in range(NCORES):
        b, h = c // 2, c % 2
        bk = out.results[c]["bucket"].astype(np.float32)  # [128 colors, 2048 t]
        tid = tgt_ids[b, h * ROWS:(h + 1) * ROWS]
        rows = slice(h * ROWS, (h + 1) * ROWS)
        numer[b, rows] = np.where(
            tid >= 0, bk[np.clip(tid, 0, KVALID - 1), np.arange(ROWS)], 0.0
        )
        denom[b, rows] = bk[0:KVALID, :].sum(axis=0)

    # host finalize, mirroring the reference ops in f32 (touches 16K scalars)
    p_gt = numer / denom
    nll = -np.log(p_gt + np.float32(EPS))
    mvalid = (numer > 0).astype(np.float32)
    nll3 = nll.reshape(B, S_TGT, L_TGT)
    m3 = mvalid.reshape(B, S_TGT, L_TGT)
    nvalid = m3.sum(-1)
    seg_loss = np.where(
        nvalid > 0, (nll3 * m3).sum(-1) / np.maximum(nvalid, np.float32(1.0)), 0.0
    ).astype(np.float32)
    cnt = int((nvalid > 0).sum())
    total = np.float32(seg_loss.sum(dtype=np.float32) / np.float32(max(cnt, 1)))
    return np.asarray(total, np.float32), np.asarray(cnt, np.int32)
